# revision 81
# baseline (speedup 1.0000x reference)
"""Trainium2 Bass kernel for nn_MultiHeadDotProductAttention_14980845928960.

Block-local multi-head attention with partial RoPE:
  q/k/v projections -> RoPE on first 32 of 64 head dims -> softmax(QK^T/8)V
  -> output projection.  Shapes: inputs [4,16,256,1024], 16 heads x 64 dim,
  blocks of 256 tokens attend locally.

Strategy: data-parallel over the 64 (batch, block) pairs -> 8 blocks/core.
Projections are batched over PAIRS of blocks (512 tokens -> N=512 moving
operands); attention runs per 256-token block.
  - inputs arrive HOST-PRE-TRANSPOSED as x^T bf16 chunks [128f, 512tok]
    (two [128, 2048] DMAs per input per pair) -- no PE transposes, no
    PSUM->SBUF cast traffic, half the input DMA bytes.
  - Q/K channel-PERMUTED (host side) so rope dims occupy out-chunks 0-3
    and pass dims chunks 4-7; RoPE = R-matmul (pair swap w/ signs) + two
    elementwise multiplies with cos/sin tables (host-precomputed bf16);
    rope emission deferred one oc-group to avoid PE head-of-line stalls.
  - scores computed TRANSPOSED (k on partitions) via 32-row tile_position
    packed matmuls (4 heads concurrent); both kc chunks column-packed per
    PSUM bank so one ScalarE exp serves two score tiles; softmax needs no
    max-subtraction (scores ~N(0,1)); denominators arrive replicated on
    PV-output partitions 64:128 via v_aug = [v_h | 1 x64]; one LUT
    reciprocal per head-pair ([64,512]); normalization folds into the
    attn PSUM->SBUF evacuation.
  - compute dtype bf16 (weights/x^T/q/k/P/v/attn), fp32 PSUM accumulate.
  - EMISSION IS SOFTWARE-PIPELINED across pairs (engine queues are FIFO):
    qk(p) | lateout(p-2) | PV(p-1) qh0 (8 aps tiles = 8 PSUM banks,
    unstalled) | PV qh1 interleaved with V-proj(p) units (no scalar ops ->
    recips batch, no ACT table thrash) | scores/exp(p) interleaved with
    out(p-1) + remaining V units (full-array matmul cover keeps the PE
    HAM clock-gate open through the ScalarE-paced score phase).
  - startup: pair-0 inputs + wq quarters race down two DMA rings ahead of
    the other weights; pair-0 projections run contraction-chunk-major so
    the first matmul needs only the first weight quarter; pair-0 evacs on
    the (then-idle) ScalarE.
All scaling (1/sqrt(D)) and biases fold into host-prepped weights
(bq,bk folded on evac; bv,bo folded as bo_eff = bo + bv @ Wo since
softmax rows sum to one).

Measured on trn2 (8 cores): ~344 us HW exec, max rel err 7.4e-3
(baseline at session start: 555-638 us). Note: sustained back-to-back
benchmarking drives the chip into P0 downclock (PE 2.4->2.0 GHz, ~+20%
on every matmul); let it cool before trusting a measurement.
"""

import ml_dtypes
import numpy as np

import concourse.bass as bass
import concourse.tile as tile
from concourse import mybir
from concourse.bass_utils import run_bass_kernel_spmd

# ---------------------------------------------------------------- constants
B, NB, BS, F = 4, 16, 256, 1024
H, D, ROPE = 16, 64, 32
NCORES = 8
BLKS = B * NB                 # 64 blocks total
BPC = BLKS // NCORES          # 8 blocks per core
NPAIR = BPC // 2              # block pairs per core
BT = 2 * BS                   # tokens per pair (512)
F32 = mybir.dt.float32
BF16 = mybir.dt.bfloat16
WDT = BF16
WNP = ml_dtypes.bfloat16
MULT = mybir.AluOpType.mult
ADD = mybir.AluOpType.add
EXP = mybir.ActivationFunctionType.Exp
LN = mybir.ActivationFunctionType.Ln
IDENT = mybir.ActivationFunctionType.Identity
def _act_reciprocal(nc, out, in_):
    # ScalarE LUT reciprocal (~1.2e-5 rel) -- bass's guard prefers DVE
    # reciprocal, which is 3.3x slower; emit directly.
    eng = nc.scalar
    return eng.add_instruction(
        mybir.InstActivation(
            name=nc.get_next_instruction_name(),
            func=mybir.ActivationFunctionType.Reciprocal,
            ins=[
                eng.lower_ap(in_),
                mybir.ImmediateValue(dtype=F32, value=0.0),
                mybir.ImmediateValue(dtype=F32, value=1.0),
                mybir.ImmediateValue(dtype=F32, value=0.0),
            ],
            outs=[eng.lower_ap(out)],
        )
    )

# ------------------------------------------------- walrus multi-wait splitter
# This walrus build rejects >1 sync-wait per instruction on several
# instruction structs. Tile attaches several waits to one instruction;
# hoist extras onto NOPs inserted just before it on the same engine.
_split_ctr = [0]


def _split_multi_waits(nc, maxw=1):
    for f in nc.m.functions:
        for bb in f.blocks:
            insts = list(bb.instructions)
            out = []
            changed = False
            for inst in insts:
                si = inst.sync_info
                waits = list(si.on_wait) if si and si.on_wait else []
                if len(waits) > maxw:
                    changed = True
                    for w in waits[:-maxw]:
                        _split_ctr[0] += 1
                        nop = mybir.InstNoOp(
                            name=f"wsplit-{_split_ctr[0]}",
                            ins=[],
                            outs=[],
                            engine=inst.engine,
                        )
                        nop.sync_info = mybir.SyncInfo(on_wait=[w], on_update=[])
                        nc.register_instruction(nop)
                        out.append(nop)
                    si.on_wait = waits[-maxw:]
                out.append(inst)
            if changed:
                bb.instructions = out


# ---------------------------------------------------------------- bass build
class _PairBuilder:
    """Emits one pair's work in phases so the caller can software-pipeline
    the EMISSION ORDER across pairs (engine queues are FIFO; any stall at
    the head blocks everything behind it on that engine)."""

    def __init__(self, nc, tc, pools, consts, pair, dram, io_tiles):
        self.nc = nc
        self.tc = tc
        (self.psum, self.xp, self.qk, self.ptp, self.attnp, self.outp,
         self.tabp) = pools
        (self.wq_sb, self.wk_sb, self.wv_sb, self.wo_sb, self.rt_sb,
         self.bq_sb, self.bk_sb, self.bo_sb, self.vaug) = consts
        self.pair = pair
        self.out_d = dram[3]
        self.xq_sb, self.xkv_sb, self.cs_sb = io_tiles(pair)
        self.cos_sb = self.cs_sb[:, 0:BT]
        self.sin_sb = self.cs_sb[:, BT : 2 * BT]
        self.pts = {}

    # ---- Q / K projections (channel-permuted; chunks 0-3 rope, 4-7 pass)
    # Rope emission for chunk oc is deferred until after the proj matmuls
    # of chunk oc+1, so the rt-matmul never head-of-line-blocks the PE
    # queue while VectorE computes `raw`.
    def _qk_proj(self, w_sb, b_sb, x_sb, tagpfx, c_outer):
        nc, psum, qk = self.nc, self.psum, self.qk
        outs = [None] * 8
        raws = {}
        pend = []

        def emit_rope(oc):
            raw = raws.pop(oc)
            qf = outs[oc]
            ps2 = psum.tile([128, BT], F32, tag="ps", name="ps2")
            nc.tensor.matmul(ps2, lhsT=self.rt_sb, rhs=raw, start=True, stop=True)
            qs2 = qk.tile([128, BT], WDT, tag="qs2", bufs=1)
            nc.vector.tensor_tensor(out=qs2, in0=ps2, in1=self.sin_sb, op=MULT)
            nc.gpsimd.tensor_tensor(out=qf, in0=raw, in1=self.cos_sb, op=MULT)
            nc.gpsimd.tensor_tensor(out=qf, in0=qf, in1=qs2, op=ADD)

        def evac(oc, ps):
            # rope `raw` evacs stay on VectorE (short chain to the rt
            # matmul); pass evacs go to ScalarE -- Identity is in every
            # ACT table set (no swap cost) and this keeps the VectorE
            # queue prefix short so the previous pair's attnT mults
            # complete sooner. Pair 0 puts everything on ScalarE (idle
            # at startup, VectorE is the c-outer burst bottleneck).
            qf = qk.tile([128, BT], WDT, tag=f"{tagpfx}{oc}", name=f"{tagpfx}{oc}")
            outs[oc] = qf
            if oc < 4:
                raw = qk.tile([128, BT], WDT, tag="raw", bufs=2)
                if self.pair == 0:
                    nc.scalar.activation(
                        out=raw, in_=ps, func=IDENT, bias=b_sb[:, oc : oc + 1]
                    )
                else:
                    nc.vector.tensor_scalar_add(raw, ps, b_sb[:, oc : oc + 1])
                raws[oc] = raw
                pend.append(oc)
            else:
                nc.scalar.activation(
                    out=qf, in_=ps, func=IDENT, bias=b_sb[:, oc : oc + 1]
                )

        def mm(ps, c, oc):
            nc.tensor.matmul(
                ps,
                lhsT=w_sb[c // 2][
                    :, (c % 2) * 1024 + oc * 128 : (c % 2) * 1024 + (oc + 1) * 128
                ],
                rhs=x_sb[c // 4][:, (c % 4) * BT : (c % 4 + 1) * BT],
                start=(c == 0),
                stop=(c == 7),
            )

        if c_outer:
            # startup variant: contraction-chunk-major so the first matmul
            # needs only the first weight half-tile
            pss = [
                psum.tile([128, BT], F32, tag="ps", name=f"pss{i}")
                for i in range(8)
            ]
            for c in range(8):
                for oc in range(8):
                    mm(pss[oc], c, oc)
            for oc in range(8):
                evac(oc, pss[oc])
        else:
            for oc in range(8):
                ps = psum.tile([128, BT], F32, tag="ps", name="ps")
                for c in range(8):
                    mm(ps, c, oc)
                evac(oc, ps)
                while pend and pend[0] + 1 <= oc:
                    emit_rope(pend.pop(0))
        while pend:
            emit_rope(pend.pop(0))
        return outs

    def qk_phase(self):
        c_outer = self.pair == 0
        self.qT = self._qk_proj(self.wq_sb, self.bq_sb, self.xq_sb, "q", c_outer)
        self.kT = self._qk_proj(self.wk_sb, self.bk_sb, self.xkv_sb, "k", c_outer)

    def se_units(self):
        """scoresT (layout [k, q], 32-row tile_position packing) + exp.
        Yields after each (qh, hg) group so the driver can interleave
        full-array matmul work between the ScalarE-paced score groups."""
        nc, psum = self.nc, self.psum
        qT, kT = self.qT, self.kT
        for qh in range(2):
            qsl = slice(qh * 256, (qh + 1) * 256)
            for hg in range(4):
                rc, pc = hg, 4 + hg
                # both kc chunks of one head column-packed per PSUM bank
                # (writes are sequential accumulation groups, no same-bank
                # concurrency) -> one exp serves both kc (halves ScalarE
                # instruction count)
                sps = []
                for g in range(4):
                    sps.append(
                        psum.tile([128, 512], F32, tag="ps", name="sps")
                    )
                for kc in range(2):
                    kc_g = qh * 2 + kc
                    ksl = slice(kc_g * 128, (kc_g + 1) * 128)
                    for g in range(4):
                        r0 = 32 * g
                        dst = sps[g][:, kc * 256 : (kc + 1) * 256]
                        nc.tensor.matmul(
                            dst,
                            lhsT=kT[rc][r0 : r0 + 32, ksl],
                            rhs=qT[rc][r0 : r0 + 32, qsl],
                            start=True,
                            stop=False,
                            tile_position=(r0, 0),
                        )
                        nc.tensor.matmul(
                            dst,
                            lhsT=kT[pc][r0 : r0 + 32, ksl],
                            rhs=qT[pc][r0 : r0 + 32, qsl],
                            start=False,
                            stop=True,
                            tile_position=(r0, 0),
                        )
                for g in range(4):
                    h = 4 * hg + g
                    pt = self.ptp.tile(
                        [128, 512], WDT,
                        tag=f"pt{qh}_{h}", name=f"pt{qh}_{h}",
                    )
                    nc.scalar.activation(out=pt, in_=sps[g], func=EXP)
                    self.pts[(qh, h)] = pt
                yield

    def v_units(self):
        """V projection into interleaved v_aug = [v_h | 1 x64] (128/head).
        The ones-columns replicate the softmax row-sum onto PV output
        partitions 64..127, already partition-broadcast for normalization."""
        nc, psum = self.nc, self.psum
        for kc in range(4):
            va3 = self.vaug[kc].rearrange("p (h c) -> p h c", c=128)
            for b2 in range(2):
                ps = psum.tile([128, 512], F32, tag="ps", name="vps")
                for c in range(8):
                    nc.tensor.matmul(
                        ps,
                        lhsT=self.xkv_sb[c // 4][
                            :, (c % 4) * BT + kc * 128 : (c % 4) * BT
                            + (kc + 1) * 128
                        ],
                        rhs=self.wv_sb[c // 4][
                            :, (c % 4) * 1024 + b2 * 512 : (c % 4) * 1024
                            + (b2 + 1) * 512
                        ],
                        start=(c == 0),
                        stop=(c == 7),
                    )
                nc.vector.tensor_copy(
                    out=va3[:, b2 * 8 : (b2 + 1) * 8, 0:64],
                    in_=ps.rearrange("p (h c) -> p h c", c=64),
                )
                yield

    def pv_units(self):
        """PV + recip + normalized evacuation; head pairs share a PSUM
        bank so one ScalarE reciprocal serves two heads. Yields per
        (qh, head-pair) unit for driver interleaving."""
        nc, psum = self.nc, self.psum
        self.attnT = [
            self.attnp.tile(
                [128, BT], WDT, tag=f"attnT{cc}", name=f"attnT{cc}", bufs=1
            )
            for cc in range(8)
        ]
        for qh in range(2):
            qsl = slice(qh * 256, (qh + 1) * 256)
            for hp in range(H // 2):
                aps = psum.tile([128, 512], F32, tag="ps", name="aps")
                for hh in range(2):
                    h = 2 * hp + hh
                    for kc in range(2):
                        nc.tensor.matmul(
                            aps[:, hh * 256 : (hh + 1) * 256],
                            lhsT=self.vaug[qh * 2 + kc][:, h * 128 : (h + 1) * 128],
                            rhs=self.pts[(qh, h)][
                                :, kc * 256 : (kc + 1) * 256
                            ],
                            start=(kc == 0),
                            stop=(kc == 1),
                        )
                rec_b = self.attnp.tile([64, 512], F32, tag="recip", bufs=2)
                _act_reciprocal(nc, rec_b, aps[64:128, :])
                for hh in range(2):
                    h = 2 * hp + hh
                    cc, r0 = h // 2, (h % 2) * 64
                    nc.vector.tensor_tensor(
                        out=self.attnT[cc][r0 : r0 + 64, qsl],
                        in0=aps[0:64, hh * 256 : (hh + 1) * 256],
                        in1=rec_b[:, hh * 256 : (hh + 1) * 256],
                        op=MULT,
                    )
                yield

    def out_units(self):
        nc, psum = self.nc, self.psum
        for t2 in range(4):
            ob = self.outp.tile([128, 1024], F32, tag="outsb", name="outsb")
            for n2 in range(2):
                ps = psum.tile([128, 512], F32, tag="ps", name="ops")
                for cc in range(8):
                    nc.tensor.matmul(
                        ps,
                        lhsT=self.attnT[cc][:, t2 * 128 : (t2 + 1) * 128],
                        rhs=self.wo_sb[cc // 4][
                            :, (cc % 4) * 1024 + n2 * 512 : (cc % 4) * 1024
                            + (n2 + 1) * 512
                        ],
                        start=(cc == 0),
                        stop=(cc == 7),
                    )
                nc.vector.tensor_tensor(
                    out=ob[:, n2 * 512 : (n2 + 1) * 512],
                    in0=ps,
                    in1=self.bo_sb[:, n2 * 512 : (n2 + 1) * 512],
                    op=ADD,
                )
            nc.sync.dma_start(
                out=self.out_d[
                    2 * self.pair + t2 // 2,
                    (t2 % 2) * 128 : (t2 % 2 + 1) * 128,
                    :,
                ],
                in_=ob,
            )
            yield


def build_program():
    nc = bass.Bass("TRN2")
    xq_d = nc.dram_tensor("xq", [NPAIR, 128, 8 * BT], WDT, kind="ExternalInput")
    xkv_d = nc.dram_tensor("xkv", [NPAIR, 128, 8 * BT], WDT, kind="ExternalInput")
    wq_d = nc.dram_tensor("wq", [128, 8 * F], WDT, kind="ExternalInput")
    wk_d = nc.dram_tensor("wk", [128, 8 * F], WDT, kind="ExternalInput")
    wv_d = nc.dram_tensor("wv", [128, 8 * F], WDT, kind="ExternalInput")
    wo_d = nc.dram_tensor("wo", [128, 8 * F], WDT, kind="ExternalInput")
    rt_d = nc.dram_tensor("rt", [128, 128], WDT, kind="ExternalInput")
    ones_d = nc.dram_tensor("ones", [1, 16, 64], WDT, kind="ExternalInput")
    bq_d = nc.dram_tensor("bq", [128, 8], F32, kind="ExternalInput")
    bk_d = nc.dram_tensor("bk", [128, 8], F32, kind="ExternalInput")
    bo_d = nc.dram_tensor("bo", [1, F], WDT, kind="ExternalInput")
    cs_d = nc.dram_tensor("cs", [NPAIR, 128, 2 * BT], WDT, kind="ExternalInput")
    out_d = nc.dram_tensor("out", [BPC, BS, F], F32, kind="ExternalOutput")

    with tile.TileContext(nc) as tc:
        with (
            tc.tile_pool(name="wpool", bufs=1) as wpool,
            tc.tile_pool(name="psum", bufs=8, space="PSUM") as psum,
            tc.tile_pool(name="xp", bufs=2) as xp,
            tc.tile_pool(name="qk", bufs=2) as qk,
            tc.tile_pool(name="ptp", bufs=1) as ptp,
            tc.tile_pool(name="attnp", bufs=1) as attnp,
            tc.tile_pool(name="outp", bufs=2) as outp,
            tc.tile_pool(name="tabp", bufs=2) as tabp,
        ):
            io_cache = {}

            def xhalves(src_d, pair, tagpfx):
                halves = []
                for hf in range(2):
                    t = xp.tile(
                        [128, 4 * BT], WDT,
                        tag=f"{tagpfx}{hf}", name=f"{tagpfx}{hf}_{pair}",
                    )
                    nc.sync.dma_start(
                        out=t, in_=src_d[pair, :, hf * 4 * BT : (hf + 1) * 4 * BT]
                    )
                    halves.append(t)
                return halves

            def io_tiles(pair):
                if pair not in io_cache:
                    xq_sb = xhalves(xq_d, pair, "xq")
                    cs_sb = tabp.tile([128, 2 * BT], WDT, tag="cs", name=f"cs{pair}")
                    nc.sync.dma_start(out=cs_sb, in_=cs_d[pair])
                    xkv_sb = xhalves(xkv_d, pair, "xkv")
                    io_cache[pair] = (xq_sb, xkv_sb, cs_sb)
                return io_cache[pair]

            # Weights live in separate part-tiles: chunk DMAs into one tile
            # WAW-serialize (each waits the previous transfer), separate
            # tiles pipeline freely.
            def wtiles(src_d, tagpfx, nparts, eng=None):
                ts = []
                w = 8 * F // nparts
                for hf in range(nparts):
                    t = wpool.tile(
                        [128, w], WDT, tag=f"{tagpfx}{hf}", name=f"{tagpfx}{hf}"
                    )
                    (eng or nc.sync).dma_start(
                        out=t, in_=src_d[:, hf * w : (hf + 1) * w]
                    )
                    ts.append(t)
                return ts

            # one sync-ring FIFO in need-order: pair-0 inputs and wq first
            # (critical path to the first matmul), then wk, then the
            # late-needed wv/wo/ones -- a second ring would steal HBM
            # bandwidth from the critical stream. wq/wk in quarters so the
            # c-outer startup projection fires as each chunk pair lands.
            # everything startup-critical rides the gpsimd ring: its engine
            # preamble finishes ~2us before sync's, so transfers start
            # earlier; the first matmul needs xq00 + wq quarter 0.
            xq00 = xp.tile([128, 4 * BT], WDT, tag="xq0", name="xq0_0")
            nc.gpsimd.dma_start(out=xq00, in_=xq_d[0, :, 0 : 4 * BT])
            wq_sb = wtiles(wq_d, "wq", 4, eng=nc.gpsimd)
            # small constants after the startup-critical xq00 transfer
            # (rt/bq/bk are first needed a few us after the first matmul)
            rt_sb = wpool.tile([128, 128], WDT, tag="rt")
            nc.sync.dma_start(out=rt_sb, in_=rt_d[:])
            bq_sb = wpool.tile([128, 8], F32, tag="bq")
            nc.sync.dma_start(out=bq_sb, in_=bq_d[:])
            bk_sb = wpool.tile([128, 8], F32, tag="bk")
            nc.sync.dma_start(out=bk_sb, in_=bk_d[:])
            xq01 = xp.tile([128, 4 * BT], WDT, tag="xq1", name="xq1_0")
            nc.sync.dma_start(out=xq01, in_=xq_d[0, :, 4 * BT : 8 * BT])
            cs0 = tabp.tile([128, 2 * BT], WDT, tag="cs", name="cs0")
            nc.sync.dma_start(out=cs0, in_=cs_d[0])
            xkv00 = xp.tile([128, 4 * BT], WDT, tag="xkv0", name="xkv0_0")
            nc.sync.dma_start(out=xkv00, in_=xkv_d[0, :, 0 : 4 * BT])
            xkv01 = xp.tile([128, 4 * BT], WDT, tag="xkv1", name="xkv1_0")
            nc.sync.dma_start(out=xkv01, in_=xkv_d[0, :, 4 * BT : 8 * BT])
            wk_sb = wtiles(wk_d, "wk", 4)
            io_cache[0] = ([xq00, xq01], [xkv00, xkv01], cs0)
            wv_sb = wtiles(wv_d, "wv", 2)

            vaug = []
            for kc in range(4):
                va = wpool.tile(
                    [128, 2048], WDT,
                    tag=f"vaug{kc}", name=f"vaug{kc}",
                )
                nc.sync.dma_start(
                    out=va.rearrange("p (h c) -> p h c", c=128)[:, :, 64:128],
                    in_=ones_d[:].to_broadcast([128, 16, 64]),
                )
                vaug.append(va)
            wo_sb = wtiles(wo_d, "wo", 2)
            bo_sb = wpool.tile([128, F], WDT, tag="bo")
            nc.sync.dma_start(out=bo_sb, in_=bo_d[0:1, :].to_broadcast([128, F]))

            pools = (psum, xp, qk, ptp, attnp, outp, tabp)
            consts = (
                wq_sb, wk_sb, wv_sb, wo_sb, rt_sb, bq_sb, bk_sb, bo_sb, vaug
            )
            dram = (xq_d, xkv_d, cs_d, out_d)

            # software-pipelined emission: the PE queue is FIFO, so pair
            # p's PV/out (which trail ScalarE exp/recip chains) are emitted
            # behind pair p+1's projections -- by the time the PE reaches
            # them, the scalar work is long done. Score groups (ScalarE-
            # exp-paced PSUM recycling, 32-row matmuls that HAM reads as
            # idle) are interleaved with full-array out/V-proj groups so
            # the PE never stalls and the clock gate stays open.
            def drain(g):
                if g is not None:
                    for _ in g:
                        pass

            prev = None         # builder for pair-1 (pv+out pending)
            lateout = None      # out generator of pair-2 (t2=2,3 pending)
            for pair in range(NPAIR):
                cur = _PairBuilder(nc, tc, pools, consts, pair, dram, io_tiles)
                cur.qk_phase()
                drain(lateout)                # t2=2,3 of pair-2 (PE cover)
                pv = prev.pv_units() if prev is not None else None
                out = prev.out_units() if prev is not None else None
                se, v = cur.se_units(), cur.v_units()
                if pv is not None:
                    # qh0: 8 aps tiles fit the 8 PSUM banks -> unstalled
                    for _ in range(8):
                        next(pv, None)
                    # qh1 recycles banks at ScalarE recip pace; cover with
                    # out-proj t2=0,1 (qh0 attnT evac'd by then)
                    for i in range(8):
                        next(pv, None)
                        if out is not None and i in (4, 6):
                            next(out, None)
                # scores/exp groups: the cover units never stall, so the
                # scheduler front-loads them -- spend ALL V-proj units
                # here (one per slot) to reach the ScalarE-paced tail
                for i in range(8):
                    next(se, None)
                    next(v, None)
                    next(se, None)
                drain(se)
                drain(v)
                lateout = out
                prev = cur
            drain(lateout)
            pv = prev.pv_units()
            out = prev.out_units()
            for _ in range(8):
                next(pv, None)
            for i in range(8):
                next(pv, None)
                if i in (5, 7):
                    next(out, None)
            drain(out)

    _split_multi_waits(nc)
    return nc


# ---------------------------------------------------------------- host side
def _host_prep(Wq, bq, Wk, bk, Wv, bv, Wo, bo):
    """Permute/scale weights; fold biases. Weight layout: [128 par,
    8 chunks x 1024 outcols] so one DMA loads a whole weight."""
    old_of_new = np.empty(F, np.int64)
    for h in range(H):
        old_of_new[h * ROPE : (h + 1) * ROPE] = h * D + np.arange(ROPE)
        old_of_new[512 + h * ROPE : 512 + (h + 1) * ROPE] = (
            h * D + ROPE + np.arange(ROPE)
        )
    def chunkmaj(w):            # [F, F] -> [128, 8*F] (chunk-major free dim)
        return np.ascontiguousarray(
            w.reshape(8, 128, F).transpose(1, 0, 2).reshape(128, 8 * F)
        )

    wq_flat = (Wq.reshape(F, F) / np.sqrt(D)).astype(np.float32)
    wq_p = chunkmaj(np.ascontiguousarray(wq_flat[:, old_of_new]))
    wk_flat = Wk.reshape(F, F).astype(np.float32)
    wk_p = chunkmaj(np.ascontiguousarray(wk_flat[:, old_of_new]))
    wv_c = chunkmaj(np.ascontiguousarray(Wv.reshape(F, F)))
    wo_c = chunkmaj(np.ascontiguousarray(Wo.reshape(F, F)))
    bq_p = np.ascontiguousarray(
        (bq.reshape(F) / np.sqrt(D))[old_of_new].reshape(8, 128).T
    ).astype(np.float32)
    bk_p = np.ascontiguousarray(bk.reshape(F)[old_of_new].reshape(8, 128).T).astype(
        np.float32
    )
    bo_eff = (bo + bv.reshape(F) @ Wo.reshape(F, F)).reshape(1, F).astype(np.float32)

    # R^T for rotate_every_two with signs: (R@q)[2i] = -q[2i+1]; [2i+1] = q[2i]
    R = np.zeros((128, 128), np.float32)
    for g in range(4):          # 4 heads per rope chunk, 32 rows each
        for i in range(ROPE // 2):
            R[g * 32 + 2 * i, g * 32 + 2 * i + 1] = -1.0
            R[g * 32 + 2 * i + 1, g * 32 + 2 * i] = 1.0
    rt = np.ascontiguousarray(R.T)
    return wq_p, wk_p, wv_c, wo_c, bq_p, bk_p, bo_eff, rt


def _tables_for_core(core):
    """cos|sin table [NPAIR, 128, 1024] bf16 for this core's block pairs."""
    inv_freq = 1.0 / 10000.0 ** (np.arange(0, ROPE, 2) / ROPE)
    cs_t = np.empty((NPAIR, 128, 2 * BT), np.float32)
    for p in range(NPAIR):
        for half in range(2):
            nb = (core * BPC + 2 * p + half) % NB
            pos = nb * BS + np.arange(BS, dtype=np.float64)
            ang = pos[None, :] * inv_freq[:, None]          # [16, 256]
            cpat = np.repeat(np.cos(ang), 2, axis=0)        # [32, 256]
            spat = np.repeat(np.sin(ang), 2, axis=0)
            sl = slice(half * BS, (half + 1) * BS)
            cs_t[p, :, sl] = np.tile(cpat, (4, 1))
            cs_t[p, :, BT + half * BS : BT + (half + 1) * BS] = np.tile(spat, (4, 1))
    return cs_t.astype(WNP)


def _xT_chunks(x_core):
    """[BPC, BS, F] fp32 -> [NPAIR, 128, 8*BT] bf16 (x^T chunk-major)."""
    xt = np.empty((NPAIR, 128, 8 * BT), WNP)
    for p in range(NPAIR):
        blk = x_core[2 * p : 2 * p + 2].reshape(BT, F)      # [512, 1024]
        t = blk.T.reshape(8, 128, BT).transpose(1, 0, 2)    # [128, 8, 512]
        xt[p] = t.reshape(128, 8 * BT).astype(WNP)
    return xt


_nc_cache = []


def kernel(inputs_q, inputs_kv, Wq, bq, Wk, bk, Wv, bv, Wo, bo):
    inputs_q = np.asarray(inputs_q, np.float32)
    inputs_kv = np.asarray(inputs_kv, np.float32)
    wq_p, wk_p, wv_c, wo_c, bq_p, bk_p, bo_eff, rt = _host_prep(
        np.asarray(Wq), np.asarray(bq), np.asarray(Wk), np.asarray(bk),
        np.asarray(Wv), np.asarray(bv), np.asarray(Wo), np.asarray(bo),
    )
    xq_all = inputs_q.reshape(BLKS, BS, F)
    xkv_all = inputs_kv.reshape(BLKS, BS, F)
    wq_p = wq_p.astype(WNP)
    wk_p = wk_p.astype(WNP)
    wv_c = wv_c.astype(WNP)
    wo_c = wo_c.astype(WNP)

    if not _nc_cache:
        _nc_cache.append(build_program())
    nc = _nc_cache[0]

    in_maps = []
    for core in range(NCORES):
        in_maps.append(
            {
                "xq": _xT_chunks(xq_all[core * BPC : (core + 1) * BPC]),
                "xkv": _xT_chunks(xkv_all[core * BPC : (core + 1) * BPC]),
                "wq": wq_p, "wk": wk_p, "wv": wv_c, "wo": wo_c,
                "rt": rt.astype(WNP), "bq": bq_p, "bk": bk_p,
                "bo": bo_eff.astype(WNP),
                "ones": np.ones((1, 16, 64), WNP),
                "cs": _tables_for_core(core),
            }
        )
    res = run_bass_kernel_spmd(nc, in_maps, list(range(NCORES)))
    out = np.concatenate([res.results[i]["out"] for i in range(NCORES)], axis=0)
    return out.reshape(B, NB, BS, F)


# revision 82
# speedup vs baseline: 1.0255x; 1.0255x over previous
"""Trainium2 Bass kernel for nn_MultiHeadDotProductAttention_14980845928960.

Block-local multi-head attention with partial RoPE:
  q/k/v projections -> RoPE on first 32 of 64 head dims -> softmax(QK^T/8)V
  -> output projection.  Shapes: inputs [4,16,256,1024], 16 heads x 64 dim,
  blocks of 256 tokens attend locally.

Strategy: data-parallel over the 64 (batch, block) pairs -> 8 blocks/core.
Projections are batched over PAIRS of blocks (512 tokens -> N=512 moving
operands); attention runs per 256-token block.
  - inputs arrive HOST-PRE-TRANSPOSED as x^T bf16 chunks [128f, 512tok]
    (two [128, 2048] DMAs per input per pair) -- no PE transposes, no
    PSUM->SBUF cast traffic, half the input DMA bytes.
  - Q/K channel-PERMUTED (host side) so rope dims occupy out-chunks 0-3
    and pass dims chunks 4-7; RoPE = R-matmul (pair swap w/ signs) + two
    elementwise multiplies with cos/sin tables (host-precomputed bf16);
    rope emission deferred one oc-group to avoid PE head-of-line stalls.
  - scores computed TRANSPOSED (k on partitions) via 32-row tile_position
    packed matmuls (4 heads concurrent); both kc chunks column-packed per
    PSUM bank so one ScalarE exp serves two score tiles; softmax needs no
    max-subtraction (scores ~N(0,1)); denominators arrive replicated on
    PV-output partitions 64:128 via v_aug = [v_h | 1 x64]; one LUT
    reciprocal per head-pair ([64,512]); normalization folds into the
    attn PSUM->SBUF evacuation.
  - compute dtype bf16 (weights/x^T/q/k/P/v/attn), fp32 PSUM accumulate.
  - EMISSION IS SOFTWARE-PIPELINED across pairs (engine queues are FIFO):
    qk(p) | lateout(p-2) | PV(p-1) qh0 (8 aps tiles = 8 PSUM banks,
    unstalled) | PV qh1 interleaved with V-proj(p) units (no scalar ops ->
    recips batch, no ACT table thrash) | scores/exp(p) interleaved with
    out(p-1) + remaining V units (full-array matmul cover keeps the PE
    HAM clock-gate open through the ScalarE-paced score phase).
  - startup: pair-0 inputs + wq quarters race down two DMA rings ahead of
    the other weights; pair-0 projections run contraction-chunk-major so
    the first matmul needs only the first weight quarter; pair-0 evacs on
    the (then-idle) ScalarE.
All scaling (1/sqrt(D)) and biases fold into host-prepped weights
(bq,bk folded on evac; bv,bo folded as bo_eff = bo + bv @ Wo since
softmax rows sum to one).

Measured on trn2 (8 cores): ~344 us HW exec, max rel err 7.4e-3
(baseline at session start: 555-638 us). Note: sustained back-to-back
benchmarking drives the chip into P0 downclock (PE 2.4->2.0 GHz, ~+20%
on every matmul); let it cool before trusting a measurement.
"""

import ml_dtypes
import numpy as np

import concourse.bass as bass
import concourse.tile as tile
from concourse import mybir
from concourse.bass_utils import run_bass_kernel_spmd

# ---------------------------------------------------------------- constants
B, NB, BS, F = 4, 16, 256, 1024
H, D, ROPE = 16, 64, 32
NCORES = 8
BLKS = B * NB                 # 64 blocks total
BPC = BLKS // NCORES          # 8 blocks per core
NPAIR = BPC // 2              # block pairs per core
BT = 2 * BS                   # tokens per pair (512)
F32 = mybir.dt.float32
BF16 = mybir.dt.bfloat16
WDT = BF16
WNP = ml_dtypes.bfloat16
MULT = mybir.AluOpType.mult
ADD = mybir.AluOpType.add
EXP = mybir.ActivationFunctionType.Exp
LN = mybir.ActivationFunctionType.Ln
IDENT = mybir.ActivationFunctionType.Identity
def _act_reciprocal(nc, out, in_):
    # ScalarE LUT reciprocal (~1.2e-5 rel) -- bass's guard prefers DVE
    # reciprocal, which is 3.3x slower; emit directly.
    eng = nc.scalar
    return eng.add_instruction(
        mybir.InstActivation(
            name=nc.get_next_instruction_name(),
            func=mybir.ActivationFunctionType.Reciprocal,
            ins=[
                eng.lower_ap(in_),
                mybir.ImmediateValue(dtype=F32, value=0.0),
                mybir.ImmediateValue(dtype=F32, value=1.0),
                mybir.ImmediateValue(dtype=F32, value=0.0),
            ],
            outs=[eng.lower_ap(out)],
        )
    )

# ------------------------------------------------- walrus multi-wait splitter
# This walrus build rejects >1 sync-wait per instruction on several
# instruction structs. Tile attaches several waits to one instruction;
# hoist extras onto NOPs inserted just before it on the same engine.
_split_ctr = [0]


def _split_multi_waits(nc, maxw=1):
    for f in nc.m.functions:
        for bb in f.blocks:
            insts = list(bb.instructions)
            out = []
            changed = False
            for inst in insts:
                si = inst.sync_info
                waits = list(si.on_wait) if si and si.on_wait else []
                if len(waits) > maxw:
                    changed = True
                    for w in waits[:-maxw]:
                        _split_ctr[0] += 1
                        nop = mybir.InstNoOp(
                            name=f"wsplit-{_split_ctr[0]}",
                            ins=[],
                            outs=[],
                            engine=inst.engine,
                        )
                        nop.sync_info = mybir.SyncInfo(on_wait=[w], on_update=[])
                        nc.register_instruction(nop)
                        out.append(nop)
                    si.on_wait = waits[-maxw:]
                out.append(inst)
            if changed:
                bb.instructions = out


# ---------------------------------------------------------------- bass build
class _PairBuilder:
    """Emits one pair's work in phases so the caller can software-pipeline
    the EMISSION ORDER across pairs (engine queues are FIFO; any stall at
    the head blocks everything behind it on that engine)."""

    def __init__(self, nc, tc, pools, consts, pair, dram, io_tiles):
        self.nc = nc
        self.tc = tc
        (self.psum, self.xp, self.qk, self.ptp, self.attnp, self.outp,
         self.tabp) = pools
        (self.wq_sb, self.wk_sb, self.wv_sb, self.wo_sb, self.rt_sb,
         self.bq_sb, self.bk_sb, self.bo_sb, self.vaug) = consts
        self.pair = pair
        self.out_d = dram[3]
        self.xq_sb, self.xkv_sb, self.cs_sb = io_tiles(pair)
        self.cos_sb = self.cs_sb[:, 0:BT]
        self.sin_sb = self.cs_sb[:, BT : 2 * BT]
        self.pts = {}

    # ---- Q / K projections (channel-permuted; chunks 0-3 rope, 4-7 pass)
    # Rope emission for chunk oc is deferred until after the proj matmuls
    # of chunk oc+1, so the rt-matmul never head-of-line-blocks the PE
    # queue while VectorE computes `raw`.
    def _qk_proj(self, w_sb, b_sb, x_sb, tagpfx, c_outer):
        nc, psum, qk = self.nc, self.psum, self.qk
        outs = [None] * 8
        raws = {}
        pend = []

        def emit_rope(oc):
            raw = raws.pop(oc)
            qf = outs[oc]
            ps2 = psum.tile([128, BT], F32, tag="ps", name="ps2")
            nc.tensor.matmul(ps2, lhsT=self.rt_sb, rhs=raw, start=True, stop=True)
            qs2 = qk.tile([128, BT], WDT, tag="qs2", bufs=1)
            nc.vector.tensor_tensor(out=qs2, in0=ps2, in1=self.sin_sb, op=MULT)
            nc.gpsimd.tensor_tensor(out=qf, in0=raw, in1=self.cos_sb, op=MULT)
            nc.gpsimd.tensor_tensor(out=qf, in0=qf, in1=qs2, op=ADD)

        def evac(oc, ps):
            # rope `raw` evacs stay on VectorE (short chain to the rt
            # matmul); pass evacs go to ScalarE -- Identity is in every
            # ACT table set (no swap cost) and this keeps the VectorE
            # queue prefix short so the previous pair's attnT mults
            # complete sooner. Pair 0 puts everything on ScalarE (idle
            # at startup, VectorE is the c-outer burst bottleneck).
            qf = qk.tile([128, BT], WDT, tag=f"{tagpfx}{oc}", name=f"{tagpfx}{oc}")
            outs[oc] = qf
            if oc < 4:
                raw = qk.tile([128, BT], WDT, tag="raw", bufs=2)
                if self.pair == 0:
                    nc.scalar.activation(
                        out=raw, in_=ps, func=IDENT, bias=b_sb[:, oc : oc + 1]
                    )
                else:
                    nc.vector.tensor_scalar_add(raw, ps, b_sb[:, oc : oc + 1])
                raws[oc] = raw
                pend.append(oc)
            else:
                nc.scalar.activation(
                    out=qf, in_=ps, func=IDENT, bias=b_sb[:, oc : oc + 1]
                )

        def mm(ps, c, oc):
            nc.tensor.matmul(
                ps,
                lhsT=w_sb[c // 2][
                    :, (c % 2) * 1024 + oc * 128 : (c % 2) * 1024 + (oc + 1) * 128
                ],
                rhs=x_sb[c // 4][:, (c % 4) * BT : (c % 4 + 1) * BT],
                start=(c == 0),
                stop=(c == 7),
            )

        if c_outer:
            # startup variant: contraction-chunk-major so the first matmul
            # needs only the first weight half-tile
            pss = [
                psum.tile([128, BT], F32, tag="ps", name=f"pss{i}")
                for i in range(8)
            ]
            for c in range(8):
                for oc in range(8):
                    mm(pss[oc], c, oc)
            for oc in range(8):
                evac(oc, pss[oc])
        else:
            for oc in range(8):
                ps = psum.tile([128, BT], F32, tag="ps", name="ps")
                for c in range(8):
                    mm(ps, c, oc)
                evac(oc, ps)
                while pend and pend[0] + 1 <= oc:
                    emit_rope(pend.pop(0))
        while pend:
            emit_rope(pend.pop(0))
        return outs

    def qk_phase(self):
        c_outer = self.pair == 0
        self.qT = self._qk_proj(self.wq_sb, self.bq_sb, self.xq_sb, "q", c_outer)
        self.kT = self._qk_proj(self.wk_sb, self.bk_sb, self.xkv_sb, "k", c_outer)

    def se_units(self):
        """scoresT (layout [k, q], 32-row tile_position packing) + exp.
        Yields after each (qh, hg) group so the driver can interleave
        full-array matmul work between the ScalarE-paced score groups."""
        nc, psum = self.nc, self.psum
        qT, kT = self.qT, self.kT
        for qh in range(2):
            qsl = slice(qh * 256, (qh + 1) * 256)
            for hg in range(4):
                rc, pc = hg, 4 + hg
                # both kc chunks of one head column-packed per PSUM bank
                # (writes are sequential accumulation groups, no same-bank
                # concurrency) -> one exp serves both kc (halves ScalarE
                # instruction count)
                sps = []
                for g in range(4):
                    sps.append(
                        psum.tile([128, 512], F32, tag="ps", name="sps")
                    )
                for kc in range(2):
                    kc_g = qh * 2 + kc
                    ksl = slice(kc_g * 128, (kc_g + 1) * 128)
                    for g in range(4):
                        r0 = 32 * g
                        dst = sps[g][:, kc * 256 : (kc + 1) * 256]
                        nc.tensor.matmul(
                            dst,
                            lhsT=kT[rc][r0 : r0 + 32, ksl],
                            rhs=qT[rc][r0 : r0 + 32, qsl],
                            start=True,
                            stop=False,
                            tile_position=(r0, 0),
                        )
                        nc.tensor.matmul(
                            dst,
                            lhsT=kT[pc][r0 : r0 + 32, ksl],
                            rhs=qT[pc][r0 : r0 + 32, qsl],
                            start=False,
                            stop=True,
                            tile_position=(r0, 0),
                        )
                for g in range(4):
                    h = 4 * hg + g
                    pt = self.ptp.tile(
                        [128, 512], WDT,
                        tag=f"pt{qh}_{h}", name=f"pt{qh}_{h}",
                    )
                    nc.scalar.activation(out=pt, in_=sps[g], func=EXP)
                    self.pts[(qh, h)] = pt
                yield

    def v_units(self):
        """V projection into interleaved v_aug = [v_h | 1 x64] (128/head).
        The ones-columns replicate the softmax row-sum onto PV output
        partitions 64..127, already partition-broadcast for normalization."""
        nc, psum = self.nc, self.psum
        for kc in range(4):
            va3 = self.vaug[kc].rearrange("p (h c) -> p h c", c=128)
            for b2 in range(2):
                ps = psum.tile([128, 512], F32, tag="ps", name="vps")
                for c in range(8):
                    nc.tensor.matmul(
                        ps,
                        lhsT=self.xkv_sb[c // 4][
                            :, (c % 4) * BT + kc * 128 : (c % 4) * BT
                            + (kc + 1) * 128
                        ],
                        rhs=self.wv_sb[c // 4][
                            :, (c % 4) * 1024 + b2 * 512 : (c % 4) * 1024
                            + (b2 + 1) * 512
                        ],
                        start=(c == 0),
                        stop=(c == 7),
                    )
                nc.vector.tensor_copy(
                    out=va3[:, b2 * 8 : (b2 + 1) * 8, 0:64],
                    in_=ps.rearrange("p (h c) -> p h c", c=64),
                )
                yield

    def pv_units(self):
        """PV + recip + normalized evacuation; head pairs share a PSUM
        bank so one ScalarE reciprocal serves two heads. Yields per
        (qh, head-pair) unit for driver interleaving."""
        nc, psum = self.nc, self.psum
        self.attnT = [
            self.attnp.tile(
                [128, BT], WDT, tag=f"attnT{cc}", name=f"attnT{cc}", bufs=1
            )
            for cc in range(8)
        ]
        for qh in range(2):
            qsl = slice(qh * 256, (qh + 1) * 256)
            for hp in range(H // 2):
                aps = psum.tile([128, 512], F32, tag="ps", name="aps")
                for hh in range(2):
                    h = 2 * hp + hh
                    for kc in range(2):
                        nc.tensor.matmul(
                            aps[:, hh * 256 : (hh + 1) * 256],
                            lhsT=self.vaug[qh * 2 + kc][:, h * 128 : (h + 1) * 128],
                            rhs=self.pts[(qh, h)][
                                :, kc * 256 : (kc + 1) * 256
                            ],
                            start=(kc == 0),
                            stop=(kc == 1),
                        )
                rec_b = self.attnp.tile([64, 512], F32, tag="recip", bufs=2)
                _act_reciprocal(nc, rec_b, aps[64:128, :])
                for hh in range(2):
                    h = 2 * hp + hh
                    cc, r0 = h // 2, (h % 2) * 64
                    nc.vector.tensor_tensor(
                        out=self.attnT[cc][r0 : r0 + 64, qsl],
                        in0=aps[0:64, hh * 256 : (hh + 1) * 256],
                        in1=rec_b[:, hh * 256 : (hh + 1) * 256],
                        op=MULT,
                    )
                yield

    def out_units(self):
        nc, psum = self.nc, self.psum
        for t2 in range(4):
            ob = self.outp.tile([128, 1024], F32, tag="outsb", name="outsb")
            for n2 in range(2):
                ps = psum.tile([128, 512], F32, tag="ps", name="ops")
                for cc in range(8):
                    nc.tensor.matmul(
                        ps,
                        lhsT=self.attnT[cc][:, t2 * 128 : (t2 + 1) * 128],
                        rhs=self.wo_sb[cc // 4][
                            :, (cc % 4) * 1024 + n2 * 512 : (cc % 4) * 1024
                            + (n2 + 1) * 512
                        ],
                        start=(cc == 0),
                        stop=(cc == 7),
                    )
                nc.vector.tensor_tensor(
                    out=ob[:, n2 * 512 : (n2 + 1) * 512],
                    in0=ps,
                    in1=self.bo_sb[:, n2 * 512 : (n2 + 1) * 512],
                    op=ADD,
                )
            nc.sync.dma_start(
                out=self.out_d[
                    2 * self.pair + t2 // 2,
                    (t2 % 2) * 128 : (t2 % 2 + 1) * 128,
                    :,
                ],
                in_=ob,
            )
            yield


def build_program():
    nc = bass.Bass("TRN2")
    xq_d = nc.dram_tensor("xq", [NPAIR, 128, 8 * BT], WDT, kind="ExternalInput")
    xkv_d = nc.dram_tensor("xkv", [NPAIR, 128, 8 * BT], WDT, kind="ExternalInput")
    wq_d = nc.dram_tensor("wq", [128, 8 * F], WDT, kind="ExternalInput")
    wk_d = nc.dram_tensor("wk", [128, 8 * F], WDT, kind="ExternalInput")
    wv_d = nc.dram_tensor("wv", [128, 8 * F], WDT, kind="ExternalInput")
    wo_d = nc.dram_tensor("wo", [128, 8 * F], WDT, kind="ExternalInput")
    rt_d = nc.dram_tensor("rt", [128, 128], WDT, kind="ExternalInput")
    ones_d = nc.dram_tensor("ones", [1, 16, 64], WDT, kind="ExternalInput")
    bq_d = nc.dram_tensor("bq", [128, 8], F32, kind="ExternalInput")
    bk_d = nc.dram_tensor("bk", [128, 8], F32, kind="ExternalInput")
    bo_d = nc.dram_tensor("bo", [1, F], WDT, kind="ExternalInput")
    cs_d = nc.dram_tensor("cs", [NPAIR, 128, 2 * BT], WDT, kind="ExternalInput")
    out_d = nc.dram_tensor("out", [BPC, BS, F], F32, kind="ExternalOutput")

    with tile.TileContext(nc) as tc:
        with (
            tc.tile_pool(name="wpool", bufs=1) as wpool,
            tc.tile_pool(name="psum", bufs=8, space="PSUM") as psum,
            tc.tile_pool(name="xp", bufs=2) as xp,
            tc.tile_pool(name="qk", bufs=2) as qk,
            tc.tile_pool(name="ptp", bufs=1) as ptp,
            tc.tile_pool(name="attnp", bufs=1) as attnp,
            tc.tile_pool(name="outp", bufs=2) as outp,
            tc.tile_pool(name="tabp", bufs=2) as tabp,
        ):
            io_cache = {}

            def xhalves(src_d, pair, tagpfx):
                halves = []
                for hf in range(2):
                    t = xp.tile(
                        [128, 4 * BT], WDT,
                        tag=f"{tagpfx}{hf}", name=f"{tagpfx}{hf}_{pair}",
                    )
                    nc.sync.dma_start(
                        out=t, in_=src_d[pair, :, hf * 4 * BT : (hf + 1) * 4 * BT]
                    )
                    halves.append(t)
                return halves

            def io_tiles(pair):
                if pair not in io_cache:
                    xq_sb = xhalves(xq_d, pair, "xq")
                    cs_sb = tabp.tile([128, 2 * BT], WDT, tag="cs", name=f"cs{pair}")
                    nc.sync.dma_start(out=cs_sb, in_=cs_d[pair])
                    xkv_sb = xhalves(xkv_d, pair, "xkv")
                    io_cache[pair] = (xq_sb, xkv_sb, cs_sb)
                return io_cache[pair]

            # Weights live in separate part-tiles: chunk DMAs into one tile
            # WAW-serialize (each waits the previous transfer), separate
            # tiles pipeline freely.
            def wtiles(src_d, tagpfx, nparts, eng=None):
                ts = []
                w = 8 * F // nparts
                for hf in range(nparts):
                    t = wpool.tile(
                        [128, w], WDT, tag=f"{tagpfx}{hf}", name=f"{tagpfx}{hf}"
                    )
                    (eng or nc.sync).dma_start(
                        out=t, in_=src_d[:, hf * w : (hf + 1) * w]
                    )
                    ts.append(t)
                return ts

            # one sync-ring FIFO in need-order: pair-0 inputs and wq first
            # (critical path to the first matmul), then wk, then the
            # late-needed wv/wo/ones -- a second ring would steal HBM
            # bandwidth from the critical stream. wq/wk in quarters so the
            # c-outer startup projection fires as each chunk pair lands.
            # everything startup-critical rides the gpsimd ring: its engine
            # preamble finishes ~2us before sync's, so transfers start
            # earlier; the first matmul needs xq00 + wq quarter 0.
            xq00 = xp.tile([128, 4 * BT], WDT, tag="xq0", name="xq0_0")
            nc.gpsimd.dma_start(out=xq00, in_=xq_d[0, :, 0 : 4 * BT])
            wq_sb = wtiles(wq_d, "wq", 4, eng=nc.gpsimd)
            # small constants after the startup-critical xq00 transfer
            # (rt/bq/bk are first needed a few us after the first matmul)
            rt_sb = wpool.tile([128, 128], WDT, tag="rt")
            nc.sync.dma_start(out=rt_sb, in_=rt_d[:])
            bq_sb = wpool.tile([128, 8], F32, tag="bq")
            nc.sync.dma_start(out=bq_sb, in_=bq_d[:])
            bk_sb = wpool.tile([128, 8], F32, tag="bk")
            nc.sync.dma_start(out=bk_sb, in_=bk_d[:])
            xq01 = xp.tile([128, 4 * BT], WDT, tag="xq1", name="xq1_0")
            nc.sync.dma_start(out=xq01, in_=xq_d[0, :, 4 * BT : 8 * BT])
            cs0 = tabp.tile([128, 2 * BT], WDT, tag="cs", name="cs0")
            nc.sync.dma_start(out=cs0, in_=cs_d[0])
            xkv00 = xp.tile([128, 4 * BT], WDT, tag="xkv0", name="xkv0_0")
            nc.sync.dma_start(out=xkv00, in_=xkv_d[0, :, 0 : 4 * BT])
            xkv01 = xp.tile([128, 4 * BT], WDT, tag="xkv1", name="xkv1_0")
            nc.sync.dma_start(out=xkv01, in_=xkv_d[0, :, 4 * BT : 8 * BT])
            wk_sb = wtiles(wk_d, "wk", 4)
            io_cache[0] = ([xq00, xq01], [xkv00, xkv01], cs0)
            wv_sb = wtiles(wv_d, "wv", 2)

            vaug = []
            for kc in range(4):
                va = wpool.tile(
                    [128, 2048], WDT,
                    tag=f"vaug{kc}", name=f"vaug{kc}",
                )
                nc.sync.dma_start(
                    out=va.rearrange("p (h c) -> p h c", c=128)[:, :, 64:128],
                    in_=ones_d[:].to_broadcast([128, 16, 64]),
                )
                vaug.append(va)
            wo_sb = wtiles(wo_d, "wo", 2)
            bo_sb = wpool.tile([128, F], WDT, tag="bo")
            nc.sync.dma_start(out=bo_sb, in_=bo_d[0:1, :].to_broadcast([128, F]))

            pools = (psum, xp, qk, ptp, attnp, outp, tabp)
            consts = (
                wq_sb, wk_sb, wv_sb, wo_sb, rt_sb, bq_sb, bk_sb, bo_sb, vaug
            )
            dram = (xq_d, xkv_d, cs_d, out_d)

            # software-pipelined emission: the PE queue is FIFO, so pair
            # p's PV/out (which trail ScalarE exp/recip chains) are emitted
            # behind pair p+1's projections -- by the time the PE reaches
            # them, the scalar work is long done. Score groups (ScalarE-
            # exp-paced PSUM recycling, 32-row matmuls that HAM reads as
            # idle) are interleaved with full-array out/V-proj groups so
            # the PE never stalls and the clock gate stays open.
            def drain(g):
                if g is not None:
                    for _ in g:
                        pass

            prev = None         # builder for pair-1 (pv+out pending)
            lateout = None      # out generator of pair-2 (t2=2,3 pending)
            for pair in range(NPAIR):
                cur = _PairBuilder(nc, tc, pools, consts, pair, dram, io_tiles)
                cur.qk_phase()
                drain(lateout)                # t2=2,3 of pair-2 (PE cover)
                pv = prev.pv_units() if prev is not None else None
                out = prev.out_units() if prev is not None else None
                se, v = cur.se_units(), cur.v_units()
                if pv is not None:
                    # qh0: 8 aps tiles fit the 8 PSUM banks -> unstalled
                    for _ in range(8):
                        next(pv, None)
                    # qh1 recycles banks at ScalarE recip pace; cover with
                    # out-proj t2=0,1 (qh0 attnT evac'd by then)
                    for i in range(8):
                        next(pv, None)
                        if out is not None and i in (5, 7):
                            next(out, None)
                # scores/exp groups: the cover units never stall, so the
                # scheduler front-loads them -- spend ALL V-proj units
                # here (one per slot) to reach the ScalarE-paced tail
                for i in range(8):
                    next(se, None)
                    next(v, None)
                    next(se, None)
                drain(se)
                drain(v)
                lateout = out
                prev = cur
            drain(lateout)
            pv = prev.pv_units()
            out = prev.out_units()
            for _ in range(8):
                next(pv, None)
            for i in range(8):
                next(pv, None)
                if i in (5, 7):
                    next(out, None)
            drain(out)

    _split_multi_waits(nc)
    return nc


# ---------------------------------------------------------------- host side
def _host_prep(Wq, bq, Wk, bk, Wv, bv, Wo, bo):
    """Permute/scale weights; fold biases. Weight layout: [128 par,
    8 chunks x 1024 outcols] so one DMA loads a whole weight."""
    old_of_new = np.empty(F, np.int64)
    for h in range(H):
        old_of_new[h * ROPE : (h + 1) * ROPE] = h * D + np.arange(ROPE)
        old_of_new[512 + h * ROPE : 512 + (h + 1) * ROPE] = (
            h * D + ROPE + np.arange(ROPE)
        )
    def chunkmaj(w):            # [F, F] -> [128, 8*F] (chunk-major free dim)
        return np.ascontiguousarray(
            w.reshape(8, 128, F).transpose(1, 0, 2).reshape(128, 8 * F)
        )

    wq_flat = (Wq.reshape(F, F) / np.sqrt(D)).astype(np.float32)
    wq_p = chunkmaj(np.ascontiguousarray(wq_flat[:, old_of_new]))
    wk_flat = Wk.reshape(F, F).astype(np.float32)
    wk_p = chunkmaj(np.ascontiguousarray(wk_flat[:, old_of_new]))
    wv_c = chunkmaj(np.ascontiguousarray(Wv.reshape(F, F)))
    wo_c = chunkmaj(np.ascontiguousarray(Wo.reshape(F, F)))
    bq_p = np.ascontiguousarray(
        (bq.reshape(F) / np.sqrt(D))[old_of_new].reshape(8, 128).T
    ).astype(np.float32)
    bk_p = np.ascontiguousarray(bk.reshape(F)[old_of_new].reshape(8, 128).T).astype(
        np.float32
    )
    bo_eff = (bo + bv.reshape(F) @ Wo.reshape(F, F)).reshape(1, F).astype(np.float32)

    # R^T for rotate_every_two with signs: (R@q)[2i] = -q[2i+1]; [2i+1] = q[2i]
    R = np.zeros((128, 128), np.float32)
    for g in range(4):          # 4 heads per rope chunk, 32 rows each
        for i in range(ROPE // 2):
            R[g * 32 + 2 * i, g * 32 + 2 * i + 1] = -1.0
            R[g * 32 + 2 * i + 1, g * 32 + 2 * i] = 1.0
    rt = np.ascontiguousarray(R.T)
    return wq_p, wk_p, wv_c, wo_c, bq_p, bk_p, bo_eff, rt


def _tables_for_core(core):
    """cos|sin table [NPAIR, 128, 1024] bf16 for this core's block pairs."""
    inv_freq = 1.0 / 10000.0 ** (np.arange(0, ROPE, 2) / ROPE)
    cs_t = np.empty((NPAIR, 128, 2 * BT), np.float32)
    for p in range(NPAIR):
        for half in range(2):
            nb = (core * BPC + 2 * p + half) % NB
            pos = nb * BS + np.arange(BS, dtype=np.float64)
            ang = pos[None, :] * inv_freq[:, None]          # [16, 256]
            cpat = np.repeat(np.cos(ang), 2, axis=0)        # [32, 256]
            spat = np.repeat(np.sin(ang), 2, axis=0)
            sl = slice(half * BS, (half + 1) * BS)
            cs_t[p, :, sl] = np.tile(cpat, (4, 1))
            cs_t[p, :, BT + half * BS : BT + (half + 1) * BS] = np.tile(spat, (4, 1))
    return cs_t.astype(WNP)


def _xT_chunks(x_core):
    """[BPC, BS, F] fp32 -> [NPAIR, 128, 8*BT] bf16 (x^T chunk-major)."""
    xt = np.empty((NPAIR, 128, 8 * BT), WNP)
    for p in range(NPAIR):
        blk = x_core[2 * p : 2 * p + 2].reshape(BT, F)      # [512, 1024]
        t = blk.T.reshape(8, 128, BT).transpose(1, 0, 2)    # [128, 8, 512]
        xt[p] = t.reshape(128, 8 * BT).astype(WNP)
    return xt


_nc_cache = []


def kernel(inputs_q, inputs_kv, Wq, bq, Wk, bk, Wv, bv, Wo, bo):
    inputs_q = np.asarray(inputs_q, np.float32)
    inputs_kv = np.asarray(inputs_kv, np.float32)
    wq_p, wk_p, wv_c, wo_c, bq_p, bk_p, bo_eff, rt = _host_prep(
        np.asarray(Wq), np.asarray(bq), np.asarray(Wk), np.asarray(bk),
        np.asarray(Wv), np.asarray(bv), np.asarray(Wo), np.asarray(bo),
    )
    xq_all = inputs_q.reshape(BLKS, BS, F)
    xkv_all = inputs_kv.reshape(BLKS, BS, F)
    wq_p = wq_p.astype(WNP)
    wk_p = wk_p.astype(WNP)
    wv_c = wv_c.astype(WNP)
    wo_c = wo_c.astype(WNP)

    if not _nc_cache:
        _nc_cache.append(build_program())
    nc = _nc_cache[0]

    in_maps = []
    for core in range(NCORES):
        in_maps.append(
            {
                "xq": _xT_chunks(xq_all[core * BPC : (core + 1) * BPC]),
                "xkv": _xT_chunks(xkv_all[core * BPC : (core + 1) * BPC]),
                "wq": wq_p, "wk": wk_p, "wv": wv_c, "wo": wo_c,
                "rt": rt.astype(WNP), "bq": bq_p, "bk": bk_p,
                "bo": bo_eff.astype(WNP),
                "ones": np.ones((1, 16, 64), WNP),
                "cs": _tables_for_core(core),
            }
        )
    res = run_bass_kernel_spmd(nc, in_maps, list(range(NCORES)))
    out = np.concatenate([res.results[i]["out"] for i in range(NCORES)], axis=0)
    return out.reshape(B, NB, BS, F)


# revision 83
# speedup vs baseline: 1.0344x; 1.0086x over previous
"""Trainium2 Bass kernel for nn_MultiHeadDotProductAttention_14980845928960.

Block-local multi-head attention with partial RoPE:
  q/k/v projections -> RoPE on first 32 of 64 head dims -> softmax(QK^T/8)V
  -> output projection.  Shapes: inputs [4,16,256,1024], 16 heads x 64 dim,
  blocks of 256 tokens attend locally.

Strategy: data-parallel over the 64 (batch, block) pairs -> 8 blocks/core.
Projections are batched over PAIRS of blocks (512 tokens -> N=512 moving
operands); attention runs per 256-token block.
  - inputs arrive HOST-PRE-TRANSPOSED as x^T bf16 chunks [128f, 512tok]
    (two [128, 2048] DMAs per input per pair) -- no PE transposes, no
    PSUM->SBUF cast traffic, half the input DMA bytes.
  - Q/K channel-PERMUTED (host side) so rope dims occupy out-chunks 0-3
    and pass dims chunks 4-7; RoPE = R-matmul (pair swap w/ signs) + two
    elementwise multiplies with cos/sin tables (host-precomputed bf16);
    rope emission deferred one oc-group to avoid PE head-of-line stalls.
  - scores computed TRANSPOSED (k on partitions) via 32-row tile_position
    packed matmuls (4 heads concurrent); both kc chunks column-packed per
    PSUM bank so one ScalarE exp serves two score tiles; softmax needs no
    max-subtraction (scores ~N(0,1)); denominators arrive replicated on
    PV-output partitions 64:128 via v_aug = [v_h | 1 x64]; one LUT
    reciprocal per head-pair ([64,512]); normalization folds into the
    attn PSUM->SBUF evacuation.
  - compute dtype bf16 (weights/x^T/q/k/P/v/attn), fp32 PSUM accumulate.
  - EMISSION IS SOFTWARE-PIPELINED across pairs (engine queues are FIFO):
    qk(p) | lateout(p-2) | PV(p-1) qh0 (8 aps tiles = 8 PSUM banks,
    unstalled) | PV qh1 interleaved with V-proj(p) units (no scalar ops ->
    recips batch, no ACT table thrash) | scores/exp(p) interleaved with
    out(p-1) + remaining V units (full-array matmul cover keeps the PE
    HAM clock-gate open through the ScalarE-paced score phase).
  - startup: pair-0 inputs + wq quarters race down two DMA rings ahead of
    the other weights; pair-0 projections run contraction-chunk-major so
    the first matmul needs only the first weight quarter; pair-0 evacs on
    the (then-idle) ScalarE.
All scaling (1/sqrt(D)) and biases fold into host-prepped weights
(bq,bk folded on evac; bv,bo folded as bo_eff = bo + bv @ Wo since
softmax rows sum to one).

Measured on trn2 (8 cores): ~344 us HW exec, max rel err 7.4e-3
(baseline at session start: 555-638 us). Note: sustained back-to-back
benchmarking drives the chip into P0 downclock (PE 2.4->2.0 GHz, ~+20%
on every matmul); let it cool before trusting a measurement.
"""

import ml_dtypes
import numpy as np

import concourse.bass as bass
import concourse.tile as tile
from concourse import mybir
from concourse.bass_utils import run_bass_kernel_spmd

# ---------------------------------------------------------------- constants
B, NB, BS, F = 4, 16, 256, 1024
H, D, ROPE = 16, 64, 32
NCORES = 8
BLKS = B * NB                 # 64 blocks total
BPC = BLKS // NCORES          # 8 blocks per core
NPAIR = BPC // 2              # block pairs per core
BT = 2 * BS                   # tokens per pair (512)
F32 = mybir.dt.float32
BF16 = mybir.dt.bfloat16
WDT = BF16
WNP = ml_dtypes.bfloat16
MULT = mybir.AluOpType.mult
ADD = mybir.AluOpType.add
EXP = mybir.ActivationFunctionType.Exp
LN = mybir.ActivationFunctionType.Ln
IDENT = mybir.ActivationFunctionType.Identity
def _act_reciprocal(nc, out, in_):
    # ScalarE LUT reciprocal (~1.2e-5 rel) -- bass's guard prefers DVE
    # reciprocal, which is 3.3x slower; emit directly.
    eng = nc.scalar
    return eng.add_instruction(
        mybir.InstActivation(
            name=nc.get_next_instruction_name(),
            func=mybir.ActivationFunctionType.Reciprocal,
            ins=[
                eng.lower_ap(in_),
                mybir.ImmediateValue(dtype=F32, value=0.0),
                mybir.ImmediateValue(dtype=F32, value=1.0),
                mybir.ImmediateValue(dtype=F32, value=0.0),
            ],
            outs=[eng.lower_ap(out)],
        )
    )

# ------------------------------------------------- walrus multi-wait splitter
# This walrus build rejects >1 sync-wait per instruction on several
# instruction structs. Tile attaches several waits to one instruction;
# hoist extras onto NOPs inserted just before it on the same engine.
_split_ctr = [0]


def _split_multi_waits(nc, maxw=1):
    for f in nc.m.functions:
        for bb in f.blocks:
            insts = list(bb.instructions)
            out = []
            changed = False
            for inst in insts:
                si = inst.sync_info
                waits = list(si.on_wait) if si and si.on_wait else []
                if len(waits) > maxw:
                    changed = True
                    for w in waits[:-maxw]:
                        _split_ctr[0] += 1
                        nop = mybir.InstNoOp(
                            name=f"wsplit-{_split_ctr[0]}",
                            ins=[],
                            outs=[],
                            engine=inst.engine,
                        )
                        nop.sync_info = mybir.SyncInfo(on_wait=[w], on_update=[])
                        nc.register_instruction(nop)
                        out.append(nop)
                    si.on_wait = waits[-maxw:]
                out.append(inst)
            if changed:
                bb.instructions = out


# ---------------------------------------------------------------- bass build
class _PairBuilder:
    """Emits one pair's work in phases so the caller can software-pipeline
    the EMISSION ORDER across pairs (engine queues are FIFO; any stall at
    the head blocks everything behind it on that engine)."""

    def __init__(self, nc, tc, pools, consts, pair, dram, io_tiles):
        self.nc = nc
        self.tc = tc
        (self.psum, self.xp, self.qk, self.ptp, self.attnp, self.outp,
         self.tabp) = pools
        (self.wq_sb, self.wk_sb, self.wv_sb, self.wo_sb, self.rt_sb,
         self.bq_sb, self.bk_sb, self.bo_sb, self.vaug) = consts
        self.pair = pair
        self.out_d = dram[3]
        self.xq_sb, self.xkv_sb, self.cs_sb = io_tiles(pair)
        self.cos_sb = self.cs_sb[:, 0:BT]
        self.sin_sb = self.cs_sb[:, BT : 2 * BT]
        self.pts = {}

    # ---- Q / K projections (channel-permuted; chunks 0-3 rope, 4-7 pass)
    # Rope emission for chunk oc is deferred until after the proj matmuls
    # of chunk oc+1, so the rt-matmul never head-of-line-blocks the PE
    # queue while VectorE computes `raw`.
    def _qk_proj(self, w_sb, b_sb, x_sb, tagpfx, c_outer):
        nc, psum, qk = self.nc, self.psum, self.qk
        outs = [None] * 8
        raws = {}
        pend = []

        def emit_rope(oc):
            raw = raws.pop(oc)
            qf = outs[oc]
            ps2 = psum.tile([128, BT], F32, tag="ps", name="ps2")
            nc.tensor.matmul(ps2, lhsT=self.rt_sb, rhs=raw, start=True, stop=True)
            qs2 = qk.tile([128, BT], WDT, tag="qs2", bufs=1)
            nc.vector.tensor_tensor(out=qs2, in0=ps2, in1=self.sin_sb, op=MULT)
            nc.gpsimd.tensor_tensor(out=qf, in0=raw, in1=self.cos_sb, op=MULT)
            nc.gpsimd.tensor_tensor(out=qf, in0=qf, in1=qs2, op=ADD)

        def evac(oc, ps):
            # rope `raw` evacs stay on VectorE (short chain to the rt
            # matmul); pass evacs go to ScalarE -- Identity is in every
            # ACT table set (no swap cost) and this keeps the VectorE
            # queue prefix short so the previous pair's attnT mults
            # complete sooner. Pair 0 puts everything on ScalarE (idle
            # at startup, VectorE is the c-outer burst bottleneck).
            qf = qk.tile([128, BT], WDT, tag=f"{tagpfx}{oc}", name=f"{tagpfx}{oc}")
            outs[oc] = qf
            if oc < 4:
                raw = qk.tile([128, BT], WDT, tag="raw", bufs=2)
                if self.pair == 0:
                    nc.scalar.activation(
                        out=raw, in_=ps, func=IDENT, bias=b_sb[:, oc : oc + 1]
                    )
                else:
                    nc.vector.tensor_scalar_add(raw, ps, b_sb[:, oc : oc + 1])
                raws[oc] = raw
                pend.append(oc)
            else:
                nc.scalar.activation(
                    out=qf, in_=ps, func=IDENT, bias=b_sb[:, oc : oc + 1]
                )

        def mm(ps, c, oc):
            nc.tensor.matmul(
                ps,
                lhsT=w_sb[c // 2][
                    :, (c % 2) * 1024 + oc * 128 : (c % 2) * 1024 + (oc + 1) * 128
                ],
                rhs=x_sb[c // 4][:, (c % 4) * BT : (c % 4 + 1) * BT],
                start=(c == 0),
                stop=(c == 7),
            )

        if c_outer:
            # startup variant: contraction-chunk-major so the first matmul
            # needs only the first weight half-tile
            pss = [
                psum.tile([128, BT], F32, tag="ps", name=f"pss{i}")
                for i in range(8)
            ]
            for c in range(8):
                for oc in range(8):
                    mm(pss[oc], c, oc)
            for oc in range(8):
                evac(oc, pss[oc])
        else:
            for oc in range(8):
                ps = psum.tile([128, BT], F32, tag="ps", name="ps")
                for c in range(8):
                    mm(ps, c, oc)
                evac(oc, ps)
                while pend and pend[0] + 1 <= oc:
                    emit_rope(pend.pop(0))
        while pend:
            emit_rope(pend.pop(0))
        return outs

    def qk_phase(self):
        c_outer = self.pair == 0
        self.qT = self._qk_proj(self.wq_sb, self.bq_sb, self.xq_sb, "q", c_outer)
        self.kT = self._qk_proj(self.wk_sb, self.bk_sb, self.xkv_sb, "k", c_outer)

    def se_units(self):
        """scoresT (layout [k, q], 32-row tile_position packing) + exp.
        Yields after each (qh, hg) group so the driver can interleave
        full-array matmul work between the ScalarE-paced score groups."""
        nc, psum = self.nc, self.psum
        qT, kT = self.qT, self.kT
        for qh in range(2):
            qsl = slice(qh * 256, (qh + 1) * 256)
            for hg in range(4):
                rc, pc = hg, 4 + hg
                # both kc chunks of one head column-packed per PSUM bank
                # (writes are sequential accumulation groups, no same-bank
                # concurrency) -> one exp serves both kc (halves ScalarE
                # instruction count)
                sps = []
                for g in range(4):
                    sps.append(
                        psum.tile([128, 512], F32, tag="ps", name="sps")
                    )
                for kc in range(2):
                    kc_g = qh * 2 + kc
                    ksl = slice(kc_g * 128, (kc_g + 1) * 128)
                    for g in range(4):
                        r0 = 32 * g
                        dst = sps[g][:, kc * 256 : (kc + 1) * 256]
                        nc.tensor.matmul(
                            dst,
                            lhsT=kT[rc][r0 : r0 + 32, ksl],
                            rhs=qT[rc][r0 : r0 + 32, qsl],
                            start=True,
                            stop=False,
                            tile_position=(r0, 0),
                        )
                        nc.tensor.matmul(
                            dst,
                            lhsT=kT[pc][r0 : r0 + 32, ksl],
                            rhs=qT[pc][r0 : r0 + 32, qsl],
                            start=False,
                            stop=True,
                            tile_position=(r0, 0),
                        )
                for g in range(4):
                    h = 4 * hg + g
                    pt = self.ptp.tile(
                        [128, 512], WDT,
                        tag=f"pt{qh}_{h}", name=f"pt{qh}_{h}",
                    )
                    nc.scalar.activation(out=pt, in_=sps[g], func=EXP)
                    self.pts[(qh, h)] = pt
                yield

    def v_units(self):
        """V projection into interleaved v_aug = [v_h | 1 x64] (128/head).
        The ones-columns replicate the softmax row-sum onto PV output
        partitions 64..127, already partition-broadcast for normalization."""
        nc, psum = self.nc, self.psum
        for kc in range(4):
            va3 = self.vaug[kc].rearrange("p (h c) -> p h c", c=128)
            for b2 in range(2):
                ps = psum.tile([128, 512], F32, tag="ps", name="vps")
                for c in range(8):
                    nc.tensor.matmul(
                        ps,
                        lhsT=self.xkv_sb[c // 4][
                            :, (c % 4) * BT + kc * 128 : (c % 4) * BT
                            + (kc + 1) * 128
                        ],
                        rhs=self.wv_sb[c // 4][
                            :, (c % 4) * 1024 + b2 * 512 : (c % 4) * 1024
                            + (b2 + 1) * 512
                        ],
                        start=(c == 0),
                        stop=(c == 7),
                    )
                nc.vector.tensor_copy(
                    out=va3[:, b2 * 8 : (b2 + 1) * 8, 0:64],
                    in_=ps.rearrange("p (h c) -> p h c", c=64),
                )
                yield

    def pv_units(self):
        """PV + recip + normalized evacuation; head pairs share a PSUM
        bank so one ScalarE reciprocal serves two heads. Yields per
        (qh, head-pair) unit for driver interleaving."""
        nc, psum = self.nc, self.psum
        self.attnT = [
            self.attnp.tile(
                [128, BT], WDT, tag=f"attnT{cc}", name=f"attnT{cc}", bufs=1
            )
            for cc in range(8)
        ]
        for qh in range(2):
            qsl = slice(qh * 256, (qh + 1) * 256)
            for hp in range(H // 2):
                aps = psum.tile([128, 512], F32, tag="ps", name="aps")
                for hh in range(2):
                    h = 2 * hp + hh
                    for kc in range(2):
                        nc.tensor.matmul(
                            aps[:, hh * 256 : (hh + 1) * 256],
                            lhsT=self.vaug[qh * 2 + kc][:, h * 128 : (h + 1) * 128],
                            rhs=self.pts[(qh, h)][
                                :, kc * 256 : (kc + 1) * 256
                            ],
                            start=(kc == 0),
                            stop=(kc == 1),
                        )
                rec_b = self.attnp.tile([64, 512], F32, tag="recip", bufs=2)
                _act_reciprocal(nc, rec_b, aps[64:128, :])
                for hh in range(2):
                    h = 2 * hp + hh
                    cc, r0 = h // 2, (h % 2) * 64
                    nc.vector.tensor_tensor(
                        out=self.attnT[cc][r0 : r0 + 64, qsl],
                        in0=aps[0:64, hh * 256 : (hh + 1) * 256],
                        in1=rec_b[:, hh * 256 : (hh + 1) * 256],
                        op=MULT,
                    )
                yield

    def out_units(self):
        nc, psum = self.nc, self.psum
        for t2 in range(4):
            ob = self.outp.tile([128, 1024], F32, tag="outsb", name="outsb")
            for n2 in range(2):
                ps = psum.tile([128, 512], F32, tag="ps", name="ops")
                for cc in range(8):
                    nc.tensor.matmul(
                        ps,
                        lhsT=self.attnT[cc][:, t2 * 128 : (t2 + 1) * 128],
                        rhs=self.wo_sb[cc // 4][
                            :, (cc % 4) * 1024 + n2 * 512 : (cc % 4) * 1024
                            + (n2 + 1) * 512
                        ],
                        start=(cc == 0),
                        stop=(cc == 7),
                    )
                nc.vector.tensor_tensor(
                    out=ob[:, n2 * 512 : (n2 + 1) * 512],
                    in0=ps,
                    in1=self.bo_sb[:, n2 * 512 : (n2 + 1) * 512],
                    op=ADD,
                )
                # store each half as soon as it is evac'd: the final DMA
                # of the kernel starts ~2us earlier, shortening the drain
                nc.sync.dma_start(
                    out=self.out_d[
                        2 * self.pair + t2 // 2,
                        (t2 % 2) * 128 : (t2 % 2 + 1) * 128,
                        n2 * 512 : (n2 + 1) * 512,
                    ],
                    in_=ob[:, n2 * 512 : (n2 + 1) * 512],
                )
            yield


def build_program():
    nc = bass.Bass("TRN2")
    xq_d = nc.dram_tensor("xq", [NPAIR, 128, 8 * BT], WDT, kind="ExternalInput")
    xkv_d = nc.dram_tensor("xkv", [NPAIR, 128, 8 * BT], WDT, kind="ExternalInput")
    wq_d = nc.dram_tensor("wq", [128, 8 * F], WDT, kind="ExternalInput")
    wk_d = nc.dram_tensor("wk", [128, 8 * F], WDT, kind="ExternalInput")
    wv_d = nc.dram_tensor("wv", [128, 8 * F], WDT, kind="ExternalInput")
    wo_d = nc.dram_tensor("wo", [128, 8 * F], WDT, kind="ExternalInput")
    rt_d = nc.dram_tensor("rt", [128, 128], WDT, kind="ExternalInput")
    ones_d = nc.dram_tensor("ones", [1, 16, 64], WDT, kind="ExternalInput")
    bq_d = nc.dram_tensor("bq", [128, 8], F32, kind="ExternalInput")
    bk_d = nc.dram_tensor("bk", [128, 8], F32, kind="ExternalInput")
    bo_d = nc.dram_tensor("bo", [1, F], WDT, kind="ExternalInput")
    cs_d = nc.dram_tensor("cs", [NPAIR, 128, 2 * BT], WDT, kind="ExternalInput")
    out_d = nc.dram_tensor("out", [BPC, BS, F], F32, kind="ExternalOutput")

    with tile.TileContext(nc) as tc:
        with (
            tc.tile_pool(name="wpool", bufs=1) as wpool,
            tc.tile_pool(name="psum", bufs=8, space="PSUM") as psum,
            tc.tile_pool(name="xp", bufs=2) as xp,
            tc.tile_pool(name="qk", bufs=2) as qk,
            tc.tile_pool(name="ptp", bufs=1) as ptp,
            tc.tile_pool(name="attnp", bufs=1) as attnp,
            tc.tile_pool(name="outp", bufs=2) as outp,
            tc.tile_pool(name="tabp", bufs=2) as tabp,
        ):
            io_cache = {}

            def xhalves(src_d, pair, tagpfx):
                halves = []
                for hf in range(2):
                    t = xp.tile(
                        [128, 4 * BT], WDT,
                        tag=f"{tagpfx}{hf}", name=f"{tagpfx}{hf}_{pair}",
                    )
                    nc.sync.dma_start(
                        out=t, in_=src_d[pair, :, hf * 4 * BT : (hf + 1) * 4 * BT]
                    )
                    halves.append(t)
                return halves

            def io_tiles(pair):
                if pair not in io_cache:
                    xq_sb = xhalves(xq_d, pair, "xq")
                    cs_sb = tabp.tile([128, 2 * BT], WDT, tag="cs", name=f"cs{pair}")
                    nc.sync.dma_start(out=cs_sb, in_=cs_d[pair])
                    xkv_sb = xhalves(xkv_d, pair, "xkv")
                    io_cache[pair] = (xq_sb, xkv_sb, cs_sb)
                return io_cache[pair]

            # Weights live in separate part-tiles: chunk DMAs into one tile
            # WAW-serialize (each waits the previous transfer), separate
            # tiles pipeline freely.
            def wtiles(src_d, tagpfx, nparts, eng=None):
                ts = []
                w = 8 * F // nparts
                for hf in range(nparts):
                    t = wpool.tile(
                        [128, w], WDT, tag=f"{tagpfx}{hf}", name=f"{tagpfx}{hf}"
                    )
                    (eng or nc.sync).dma_start(
                        out=t, in_=src_d[:, hf * w : (hf + 1) * w]
                    )
                    ts.append(t)
                return ts

            # one sync-ring FIFO in need-order: pair-0 inputs and wq first
            # (critical path to the first matmul), then wk, then the
            # late-needed wv/wo/ones -- a second ring would steal HBM
            # bandwidth from the critical stream. wq/wk in quarters so the
            # c-outer startup projection fires as each chunk pair lands.
            # everything startup-critical rides the gpsimd ring: its engine
            # preamble finishes ~2us before sync's, so transfers start
            # earlier; the first matmul needs xq00 + wq quarter 0.
            xq00 = xp.tile([128, 4 * BT], WDT, tag="xq0", name="xq0_0")
            nc.gpsimd.dma_start(out=xq00, in_=xq_d[0, :, 0 : 4 * BT])
            wq_sb = wtiles(wq_d, "wq", 4, eng=nc.gpsimd)
            # small constants after the startup-critical xq00 transfer
            # (rt/bq/bk are first needed a few us after the first matmul)
            rt_sb = wpool.tile([128, 128], WDT, tag="rt")
            nc.sync.dma_start(out=rt_sb, in_=rt_d[:])
            bq_sb = wpool.tile([128, 8], F32, tag="bq")
            nc.sync.dma_start(out=bq_sb, in_=bq_d[:])
            bk_sb = wpool.tile([128, 8], F32, tag="bk")
            nc.sync.dma_start(out=bk_sb, in_=bk_d[:])
            xq01 = xp.tile([128, 4 * BT], WDT, tag="xq1", name="xq1_0")
            nc.sync.dma_start(out=xq01, in_=xq_d[0, :, 4 * BT : 8 * BT])
            cs0 = tabp.tile([128, 2 * BT], WDT, tag="cs", name="cs0")
            nc.sync.dma_start(out=cs0, in_=cs_d[0])
            xkv00 = xp.tile([128, 4 * BT], WDT, tag="xkv0", name="xkv0_0")
            nc.sync.dma_start(out=xkv00, in_=xkv_d[0, :, 0 : 4 * BT])
            xkv01 = xp.tile([128, 4 * BT], WDT, tag="xkv1", name="xkv1_0")
            nc.sync.dma_start(out=xkv01, in_=xkv_d[0, :, 4 * BT : 8 * BT])
            wk_sb = wtiles(wk_d, "wk", 4)
            io_cache[0] = ([xq00, xq01], [xkv00, xkv01], cs0)
            wv_sb = wtiles(wv_d, "wv", 2)

            vaug = []
            for kc in range(4):
                va = wpool.tile(
                    [128, 2048], WDT,
                    tag=f"vaug{kc}", name=f"vaug{kc}",
                )
                nc.sync.dma_start(
                    out=va.rearrange("p (h c) -> p h c", c=128)[:, :, 64:128],
                    in_=ones_d[:].to_broadcast([128, 16, 64]),
                )
                vaug.append(va)
            wo_sb = wtiles(wo_d, "wo", 2)
            bo_sb = wpool.tile([128, F], WDT, tag="bo")
            nc.sync.dma_start(out=bo_sb, in_=bo_d[0:1, :].to_broadcast([128, F]))

            pools = (psum, xp, qk, ptp, attnp, outp, tabp)
            consts = (
                wq_sb, wk_sb, wv_sb, wo_sb, rt_sb, bq_sb, bk_sb, bo_sb, vaug
            )
            dram = (xq_d, xkv_d, cs_d, out_d)

            # software-pipelined emission: the PE queue is FIFO, so pair
            # p's PV/out (which trail ScalarE exp/recip chains) are emitted
            # behind pair p+1's projections -- by the time the PE reaches
            # them, the scalar work is long done. Score groups (ScalarE-
            # exp-paced PSUM recycling, 32-row matmuls that HAM reads as
            # idle) are interleaved with full-array out/V-proj groups so
            # the PE never stalls and the clock gate stays open.
            def drain(g):
                if g is not None:
                    for _ in g:
                        pass

            prev = None         # builder for pair-1 (pv+out pending)
            lateout = None      # out generator of pair-2 (t2=2,3 pending)
            for pair in range(NPAIR):
                cur = _PairBuilder(nc, tc, pools, consts, pair, dram, io_tiles)
                cur.qk_phase()
                drain(lateout)                # t2=2,3 of pair-2 (PE cover)
                pv = prev.pv_units() if prev is not None else None
                out = prev.out_units() if prev is not None else None
                se, v = cur.se_units(), cur.v_units()
                if pv is not None:
                    # qh0: 8 aps tiles fit the 8 PSUM banks -> unstalled
                    for _ in range(8):
                        next(pv, None)
                    # qh1 recycles banks at ScalarE recip pace; cover with
                    # out-proj t2=0,1 (qh0 attnT evac'd by then)
                    for i in range(8):
                        next(pv, None)
                        if out is not None and i in (5, 7):
                            next(out, None)
                # scores/exp groups: the cover units never stall, so the
                # scheduler front-loads them -- spend ALL V-proj units
                # here (one per slot) to reach the ScalarE-paced tail
                for i in range(8):
                    next(se, None)
                    next(v, None)
                    next(se, None)
                drain(se)
                drain(v)
                lateout = out
                prev = cur
            drain(lateout)
            pv = prev.pv_units()
            out = prev.out_units()
            for _ in range(8):
                next(pv, None)
            for i in range(8):
                next(pv, None)
                if i in (5, 7):
                    next(out, None)
            drain(out)

    _split_multi_waits(nc)
    return nc


# ---------------------------------------------------------------- host side
def _host_prep(Wq, bq, Wk, bk, Wv, bv, Wo, bo):
    """Permute/scale weights; fold biases. Weight layout: [128 par,
    8 chunks x 1024 outcols] so one DMA loads a whole weight."""
    old_of_new = np.empty(F, np.int64)
    for h in range(H):
        old_of_new[h * ROPE : (h + 1) * ROPE] = h * D + np.arange(ROPE)
        old_of_new[512 + h * ROPE : 512 + (h + 1) * ROPE] = (
            h * D + ROPE + np.arange(ROPE)
        )
    def chunkmaj(w):            # [F, F] -> [128, 8*F] (chunk-major free dim)
        return np.ascontiguousarray(
            w.reshape(8, 128, F).transpose(1, 0, 2).reshape(128, 8 * F)
        )

    wq_flat = (Wq.reshape(F, F) / np.sqrt(D)).astype(np.float32)
    wq_p = chunkmaj(np.ascontiguousarray(wq_flat[:, old_of_new]))
    wk_flat = Wk.reshape(F, F).astype(np.float32)
    wk_p = chunkmaj(np.ascontiguousarray(wk_flat[:, old_of_new]))
    wv_c = chunkmaj(np.ascontiguousarray(Wv.reshape(F, F)))
    wo_c = chunkmaj(np.ascontiguousarray(Wo.reshape(F, F)))
    bq_p = np.ascontiguousarray(
        (bq.reshape(F) / np.sqrt(D))[old_of_new].reshape(8, 128).T
    ).astype(np.float32)
    bk_p = np.ascontiguousarray(bk.reshape(F)[old_of_new].reshape(8, 128).T).astype(
        np.float32
    )
    bo_eff = (bo + bv.reshape(F) @ Wo.reshape(F, F)).reshape(1, F).astype(np.float32)

    # R^T for rotate_every_two with signs: (R@q)[2i] = -q[2i+1]; [2i+1] = q[2i]
    R = np.zeros((128, 128), np.float32)
    for g in range(4):          # 4 heads per rope chunk, 32 rows each
        for i in range(ROPE // 2):
            R[g * 32 + 2 * i, g * 32 + 2 * i + 1] = -1.0
            R[g * 32 + 2 * i + 1, g * 32 + 2 * i] = 1.0
    rt = np.ascontiguousarray(R.T)
    return wq_p, wk_p, wv_c, wo_c, bq_p, bk_p, bo_eff, rt


def _tables_for_core(core):
    """cos|sin table [NPAIR, 128, 1024] bf16 for this core's block pairs."""
    inv_freq = 1.0 / 10000.0 ** (np.arange(0, ROPE, 2) / ROPE)
    cs_t = np.empty((NPAIR, 128, 2 * BT), np.float32)
    for p in range(NPAIR):
        for half in range(2):
            nb = (core * BPC + 2 * p + half) % NB
            pos = nb * BS + np.arange(BS, dtype=np.float64)
            ang = pos[None, :] * inv_freq[:, None]          # [16, 256]
            cpat = np.repeat(np.cos(ang), 2, axis=0)        # [32, 256]
            spat = np.repeat(np.sin(ang), 2, axis=0)
            sl = slice(half * BS, (half + 1) * BS)
            cs_t[p, :, sl] = np.tile(cpat, (4, 1))
            cs_t[p, :, BT + half * BS : BT + (half + 1) * BS] = np.tile(spat, (4, 1))
    return cs_t.astype(WNP)


def _xT_chunks(x_core):
    """[BPC, BS, F] fp32 -> [NPAIR, 128, 8*BT] bf16 (x^T chunk-major)."""
    xt = np.empty((NPAIR, 128, 8 * BT), WNP)
    for p in range(NPAIR):
        blk = x_core[2 * p : 2 * p + 2].reshape(BT, F)      # [512, 1024]
        t = blk.T.reshape(8, 128, BT).transpose(1, 0, 2)    # [128, 8, 512]
        xt[p] = t.reshape(128, 8 * BT).astype(WNP)
    return xt


_nc_cache = []


def kernel(inputs_q, inputs_kv, Wq, bq, Wk, bk, Wv, bv, Wo, bo):
    inputs_q = np.asarray(inputs_q, np.float32)
    inputs_kv = np.asarray(inputs_kv, np.float32)
    wq_p, wk_p, wv_c, wo_c, bq_p, bk_p, bo_eff, rt = _host_prep(
        np.asarray(Wq), np.asarray(bq), np.asarray(Wk), np.asarray(bk),
        np.asarray(Wv), np.asarray(bv), np.asarray(Wo), np.asarray(bo),
    )
    xq_all = inputs_q.reshape(BLKS, BS, F)
    xkv_all = inputs_kv.reshape(BLKS, BS, F)
    wq_p = wq_p.astype(WNP)
    wk_p = wk_p.astype(WNP)
    wv_c = wv_c.astype(WNP)
    wo_c = wo_c.astype(WNP)

    if not _nc_cache:
        _nc_cache.append(build_program())
    nc = _nc_cache[0]

    in_maps = []
    for core in range(NCORES):
        in_maps.append(
            {
                "xq": _xT_chunks(xq_all[core * BPC : (core + 1) * BPC]),
                "xkv": _xT_chunks(xkv_all[core * BPC : (core + 1) * BPC]),
                "wq": wq_p, "wk": wk_p, "wv": wv_c, "wo": wo_c,
                "rt": rt.astype(WNP), "bq": bq_p, "bk": bk_p,
                "bo": bo_eff.astype(WNP),
                "ones": np.ones((1, 16, 64), WNP),
                "cs": _tables_for_core(core),
            }
        )
    res = run_bass_kernel_spmd(nc, in_maps, list(range(NCORES)))
    out = np.concatenate([res.results[i]["out"] for i in range(NCORES)], axis=0)
    return out.reshape(B, NB, BS, F)
